# revision 10
# baseline (speedup 1.0000x reference)
"""Multi-head self-attention (B=4, S=2048, D=1024, H=16) on 8 trn2 NeuronCores.

Sharding: core c -> batch b = c//2, head-group g = c%2 (8 heads, 512 of the
1024 output/QKV columns). Each core computes Q/K/V projections for its slice
and full attention for its 8 heads. Host does layout prep (bf16 conversion,
x transpose, W column slices) and the final gather/transpose - no collectives.

v2 vs v1: all matmuls in bf16 (v1's float32r lowered to fp32_mode=HIGH
multi-pass matmuls at ~2-4x the cost; the trace showed PE 100% busy for
680us of the 830us attn phase). Single x pass (V uses x as stationary, Q/K
as moving, same resident SBUF copy). Exp batched to 1024 free-dim per
ACTIVATE (halves the per-instruction overhead on the Scalar engine).

Per-core pipeline:
  phase 1 (qkv): V[s,dloc] psum groups (stationary=x chunk, moving=Wv) ->
           vx[128,16,8,65] bf16 with a ones column per head (PV denominator);
           KT/QT[128(2 heads x 64 dh), hp, s] bf16 via (stationary=W chunk,
           moving=x chunk) psum groups.
  phase 2 (attn): per (hp, qc): 16 k-blocks:
           scoresT pair (2 heads, tile_position row split) -> sp[128,2,512]
           psum (2 banks); one ACTIVATE Exp(scale=1/16) -> ex[128,2,512]
           bf16; 2 PV matmuls accumulate pv[65,512] (row 64 = denominator);
           then normalize: out = pv[0:64] * partition_broadcast(1/pv[64]).
"""
import ml_dtypes
import numpy as np

import concourse.bacc as bacc
import concourse.mybir as mybir
import concourse.tile as tile
from concourse.bass_utils import run_bass_kernel_spmd

B, S, D, H = 4, 2048, 1024, 16
DH = D // H            # 64
NCORES = 8
HLOC = H // 2          # 8 heads per core
DLOC = HLOC * DH       # 512 output cols per core
F32 = mybir.dt.float32
BF16 = mybir.dt.bfloat16
EXPF = mybir.ActivationFunctionType.Exp

SC = 512               # s-chunk in phase 1
NSC = S // SC          # 4
NKB = S // 128         # 16 k-blocks
NDT = D // 128         # 8 contraction tiles for QKV
NHP = HLOC // 2        # 4 head pairs


def _build():
    nc = bacc.Bacc("TRN2", target_bir_lowering=False, debug=False,
                   num_devices=NCORES)
    # x: [p, sc, sb, o, j] with d = o*128+p, s = sc*512+sb*128+j
    x_h = nc.dram_tensor("x4", [128, NSC, 4, NDT, 128], BF16,
                         kind="ExternalInput").ap()
    wq_h = nc.dram_tensor("Wq", [128, NDT, DLOC], BF16,
                          kind="ExternalInput").ap()
    wk_h = nc.dram_tensor("Wk", [128, NDT, DLOC], BF16,
                          kind="ExternalInput").ap()
    wv_h = nc.dram_tensor("Wv", [128, NDT, DLOC], BF16,
                          kind="ExternalInput").ap()
    out = nc.dram_tensor("outT", [DLOC, S], F32, kind="ExternalOutput").ap()
    out_t = out.rearrange("(o p) s -> p o s", p=128)      # [128, 4, 2048]

    with tile.TileContext(nc) as tc:
        with tc.tile_pool(name="persist", bufs=1) as keep:
            x_sb = keep.tile([128, NSC, 4, NDT, 128], BF16)
            wq_sb = keep.tile([128, NDT, DLOC], BF16)
            wk_sb = keep.tile([128, NDT, DLOC], BF16)
            wv_sb = keep.tile([128, NDT, DLOC], BF16)
            vx = keep.tile([128, NKB, HLOC, DH + 1], BF16)
            kt = keep.tile([128, NHP, S], BF16)   # [2x64 dh, hp, s]
            qt = keep.tile([128, NHP, S], BF16)
            ot = keep.tile([128, NHP, S], F32)

            nc.sync.dma_start(wv_sb[:, 0:4], wv_h[:, 0:4])
            nc.sync.dma_start(wv_sb[:, 4:8], wv_h[:, 4:8])
            for sb in range(4):
                nc.sync.dma_start(x_sb[:, 0, sb], x_h[:, 0, sb])
            nc.sync.dma_start(wk_sb[:, 0:4], wk_h[:, 0:4])
            nc.sync.dma_start(wk_sb[:, 4:8], wk_h[:, 4:8])
            for sc in range(1, NSC):
                for sb in range(4):
                    nc.sync.dma_start(x_sb[:, sc, sb], x_h[:, sc, sb])
            nc.sync.dma_start(wq_sb[:, 0:4], wq_h[:, 0:4])
            nc.sync.dma_start(wq_sb[:, 4:8], wq_h[:, 4:8])
            ones_t = keep.tile([128, NKB, HLOC], BF16)
            nc.vector.memset(ones_t[:], 1.0)
            nc.vector.tensor_copy(vx[:, :, :, DH], ones_t[:])

            with tc.tile_pool(name="p1ps", bufs=2, space="PSUM") as p1ps, \
                 tc.tile_pool(name="spp", bufs=2, space="PSUM") as spp, \
                 tc.tile_pool(name="pvp", bufs=1, space="PSUM") as pvp, \
                 tc.tile_pool(name="exp", bufs=6) as exp_pool, \
                 tc.tile_pool(name="nrm", bufs=2) as nrm:

                pending = {}

                def qk_group(hp, w_sb, dst, sc, part=None, nparts=1):
                    cs = slice(hp * 128, (hp + 1) * 128)
                    ss = slice(sc * SC, (sc + 1) * SC)
                    key = (hp, id(w_sb), sc)
                    step = NDT // nparts
                    if part is None or part == 0:
                        ps = p1ps.tile([128, SC], F32, tag="p1", name="psqk")
                        if part == 0:
                            pending[key] = ps
                        dts = range(NDT) if part is None else range(step)
                    else:
                        ps = pending[key] if part < nparts - 1                             else pending.pop(key)
                        dts = range(part * step, (part + 1) * step)
                    for dt_i in dts:
                        nc.tensor.matmul(
                            ps[:],
                            w_sb[:, dt_i, cs],
                            x_sb[:, sc, :, dt_i, :],
                            start=(dt_i == 0), stop=(dt_i == NDT - 1),
                            skip_group_check=True,
                        )
                    if part is None or part == nparts - 1:
                        nc.vector.tensor_copy(dst[:, hp, ss], ps[:])

                def v_group(sc, sb):
                    ps = p1ps.tile([128, DLOC], F32, tag="p1", name="psv")
                    for dt_i in range(NDT):
                        nc.tensor.matmul(
                            ps[:],
                            x_sb[:, sc, sb, dt_i, :],
                            wv_sb[:, dt_i, :],
                            start=(dt_i == 0), stop=(dt_i == NDT - 1),
                            skip_group_check=True,
                        )
                    s_idx = sc * (SC // 128) + sb
                    nc.vector.tensor_copy(
                        vx[:, s_idx, :, 0:DH],
                        ps[:].rearrange("p (h d) -> p h d", h=HLOC))

                # -------- phase 1 head start: V (all) + K0 (all) + Q0 ----
                with nc.named_scope("qkv"):
                    for sc in range(NSC):
                        for sb in range(SC // 128):
                            v_group(sc, sb)
                        qk_group(0, wk_sb, kt, sc)
                    qk_group(0, wq_sb, qt, 0)

                # -------- phase 2: attention (QK for hp+1 interleaved) ---
                with nc.named_scope("attn"):
                    for hp in range(NHP):
                        for qc in range(NSC):
                            qs = slice(qc * SC, (qc + 1) * SC)
                            pvs = [pvp.tile([DH + 1, SC], F32, tag=f"pv{h}",
                                            name=f"pv{h}") for h in range(2)]
                            for kb in range(NKB):
                                ks = slice(kb * 128, (kb + 1) * 128)
                                sp = spp.tile([128, 2, SC], F32, tag="sp",
                                              name="sp")
                                for h in range(2):
                                    nc.tensor.matmul(
                                        sp[:, h, :],
                                        kt[64 * h:64 * h + 64, hp, ks],
                                        qt[64 * h:64 * h + 64, hp, qs],
                                        start=True, stop=True,
                                        tile_position=(64 * h, 0))
                                ex = exp_pool.tile([128, 2, SC], BF16,
                                                   tag="ex", name="ex")
                                nc.scalar.activation(ex[:], sp[:], EXPF,
                                                     scale=1.0 / H)
                                for h in range(2):
                                    nc.tensor.matmul(
                                        pvs[h][:], vx[:, kb, 2 * hp + h, :],
                                        ex[:, h, :],
                                        start=(kb == 0), stop=(kb == NKB - 1),
                                        skip_group_check=True)
                                if hp == 0 and qc < NSC - 1 and 1 <= kb <= 4:
                                    qk_group(0, wq_sb, qt, qc + 1,
                                             part=kb - 1, nparts=4)
                                if hp < NHP - 1:
                                    if 6 <= kb <= 9:
                                        qk_group(hp + 1, wk_sb, kt, qc,
                                                 part=kb - 6, nparts=4)
                                    elif 11 <= kb <= 14:
                                        qk_group(hp + 1, wq_sb, qt, qc,
                                                 part=kb - 11, nparts=4)
                            drs, pvcs, dens, bcs = [], [], [], []
                            for h in range(2):
                                dr = nrm.tile([1, SC], F32, tag=f"dr{h}",
                                              name="dr")
                                nc.vector.tensor_copy(dr[:],
                                                      pvs[h][DH:DH + 1, :])
                                pvc = nrm.tile([DH, SC], F32, tag=f"pvc{h}",
                                               name="pvc")
                                nc.vector.tensor_copy(pvc[:],
                                                      pvs[h][0:DH, :])
                                drs.append(dr); pvcs.append(pvc)
                            for h in range(2):
                                den = nrm.tile([1, SC], F32, tag=f"den{h}",
                                               name="den")
                                nc.vector.reciprocal_approx_fast(den[:],
                                                                 drs[h][:])
                                bc = nrm.tile([DH, SC], F32, tag=f"bc{h}",
                                              name="bc")
                                nc.gpsimd.partition_broadcast(bc[:], den[:])
                                dens.append(den); bcs.append(bc)
                            for h in range(2):
                                nc.vector.tensor_mul(
                                    ot[64 * h:64 * h + 64, hp, qs],
                                    pvcs[h][:], bcs[h][:])
                            nc.sync.dma_start(out_t[:, hp, qs],
                                              ot[:, hp, qs])

    nc.compile()
    return nc


def run(inputs, trace=False):
    x = np.asarray(inputs["encoder_input"], dtype=np.float32)
    Wq = np.asarray(inputs["Wq"], dtype=np.float32)
    Wk = np.asarray(inputs["Wk"], dtype=np.float32)
    Wv = np.asarray(inputs["Wv"], dtype=np.float32)
    bf = ml_dtypes.bfloat16

    nc = _build()
    in_maps = []
    for c in range(NCORES):
        b, g = c // 2, c % 2
        cols = slice(g * DLOC, (g + 1) * DLOC)
        xT = x[b].T                                       # [1024, 2048]
        x4 = (xT.reshape(NDT, 128, NSC, 4, 128)
              .transpose(1, 2, 3, 0, 4))
        in_maps.append({
            "x4": np.ascontiguousarray(x4.astype(bf)),
            "Wq": np.ascontiguousarray(
                Wq[:, cols].reshape(NDT, 128, DLOC).transpose(1, 0, 2)
                .astype(bf)),
            "Wk": np.ascontiguousarray(
                Wk[:, cols].reshape(NDT, 128, DLOC).transpose(1, 0, 2)
                .astype(bf)),
            "Wv": np.ascontiguousarray(
                Wv[:, cols].reshape(NDT, 128, DLOC).transpose(1, 0, 2)
                .astype(bf)),
        })
    res = run_bass_kernel_spmd(nc, in_maps, core_ids=list(range(NCORES)),
                               trace=trace)
    out = np.empty((B, S, D), dtype=np.float32)
    for c in range(NCORES):
        b, g = c // 2, c % 2
        out[b, :, g * DLOC:(g + 1) * DLOC] = res.results[c]["outT"].T
    return out, res


def kernel(**inputs):
    out, _ = run(inputs, trace=False)
    return out


# revision 11
# speedup vs baseline: 1.0149x; 1.0149x over previous
"""Multi-head self-attention (B=4, S=2048, D=1024, H=16) on 8 trn2 NeuronCores.

Sharding: core c -> batch b = c//2, head-group g = c%2 (8 heads, 512 of the
1024 output/QKV columns). Each core computes Q/K/V projections for its slice
and full attention for its 8 heads. Host does layout prep (bf16 conversion,
x transpose, W column slices) and the final gather/transpose - no collectives.

v2 vs v1: all matmuls in bf16 (v1's float32r lowered to fp32_mode=HIGH
multi-pass matmuls at ~2-4x the cost; the trace showed PE 100% busy for
680us of the 830us attn phase). Single x pass (V uses x as stationary, Q/K
as moving, same resident SBUF copy). Exp batched to 1024 free-dim per
ACTIVATE (halves the per-instruction overhead on the Scalar engine).

Per-core pipeline:
  phase 1 (qkv): V[s,dloc] psum groups (stationary=x chunk, moving=Wv) ->
           vx[128,16,8,65] bf16 with a ones column per head (PV denominator);
           KT/QT[128(2 heads x 64 dh), hp, s] bf16 via (stationary=W chunk,
           moving=x chunk) psum groups.
  phase 2 (attn): per (hp, qc): 16 k-blocks:
           scoresT pair (2 heads, tile_position row split) -> sp[128,2,512]
           psum (2 banks); one ACTIVATE Exp(scale=1/16) -> ex[128,2,512]
           bf16; 2 PV matmuls accumulate pv[65,512] (row 64 = denominator);
           then normalize: out = pv[0:64] * partition_broadcast(1/pv[64]).
"""
import ml_dtypes
import numpy as np

import concourse.bacc as bacc
import concourse.mybir as mybir
import concourse.tile as tile
from concourse.bass_utils import run_bass_kernel_spmd

B, S, D, H = 4, 2048, 1024, 16
DH = D // H            # 64
NCORES = 8
HLOC = H // 2          # 8 heads per core
DLOC = HLOC * DH       # 512 output cols per core
F32 = mybir.dt.float32
BF16 = mybir.dt.bfloat16
EXPF = mybir.ActivationFunctionType.Exp

SC = 512               # s-chunk in phase 1
NSC = S // SC          # 4
NKB = S // 128         # 16 k-blocks
NDT = D // 128         # 8 contraction tiles for QKV
NHP = HLOC // 2        # 4 head pairs


def _build():
    nc = bacc.Bacc("TRN2", target_bir_lowering=False, debug=False,
                   num_devices=NCORES)
    # x: [p, sc, sb, o, j] with d = o*128+p, s = sc*512+sb*128+j
    x_h = nc.dram_tensor("x4", [128, NSC, 4, NDT, 128], BF16,
                         kind="ExternalInput").ap()
    wq_h = nc.dram_tensor("Wq", [128, NDT, DLOC], BF16,
                          kind="ExternalInput").ap()
    wk_h = nc.dram_tensor("Wk", [128, NDT, DLOC], BF16,
                          kind="ExternalInput").ap()
    wv_h = nc.dram_tensor("Wv", [128, NDT, DLOC], BF16,
                          kind="ExternalInput").ap()
    out = nc.dram_tensor("outT", [DLOC, S], F32, kind="ExternalOutput").ap()
    out_t = out.rearrange("(o p) s -> p o s", p=128)      # [128, 4, 2048]

    with tile.TileContext(nc) as tc:
        with tc.tile_pool(name="persist", bufs=1) as keep:
            x_sb = keep.tile([128, NSC, 4, NDT, 128], BF16)
            wq_sb = keep.tile([128, NDT, DLOC], BF16)
            wk_sb = keep.tile([128, NDT, DLOC], BF16)
            wv_sb = keep.tile([128, NDT, DLOC], BF16)
            vx = keep.tile([128, NKB, HLOC, DH + 1], BF16)
            kt = keep.tile([128, NHP, S], BF16)   # [2x64 dh, hp, s]
            qt = keep.tile([128, NHP, S], BF16)
            ot = keep.tile([128, NHP, S], F32)

            nc.sync.dma_start(wv_sb[:, 0:4], wv_h[:, 0:4])
            nc.sync.dma_start(wv_sb[:, 4:8], wv_h[:, 4:8])
            for sb in range(4):
                nc.sync.dma_start(x_sb[:, 0, sb], x_h[:, 0, sb])
            nc.sync.dma_start(wk_sb[:, 0:4], wk_h[:, 0:4])
            nc.sync.dma_start(wk_sb[:, 4:8], wk_h[:, 4:8])
            for sc in range(1, NSC):
                for sb in range(4):
                    nc.sync.dma_start(x_sb[:, sc, sb], x_h[:, sc, sb])
            nc.sync.dma_start(wq_sb[:, 0:4], wq_h[:, 0:4])
            nc.sync.dma_start(wq_sb[:, 4:8], wq_h[:, 4:8])
            ones_t = keep.tile([128, NKB, HLOC], BF16)
            nc.vector.memset(ones_t[:], 1.0)
            nc.vector.tensor_copy(vx[:, :, :, DH], ones_t[:])

            with tc.tile_pool(name="p1ps", bufs=2, space="PSUM") as p1ps, \
                 tc.tile_pool(name="spp", bufs=2, space="PSUM") as spp, \
                 tc.tile_pool(name="pvp", bufs=1, space="PSUM") as pvp, \
                 tc.tile_pool(name="exp", bufs=4) as exp_pool, \
                 tc.tile_pool(name="nrm", bufs=2) as nrm:

                pending = {}

                def qk_group(hp, w_sb, dst, sc, part=None, nparts=1):
                    cs = slice(hp * 128, (hp + 1) * 128)
                    ss = slice(sc * SC, (sc + 1) * SC)
                    key = (hp, id(w_sb), sc)
                    step = NDT // nparts
                    if part is None or part == 0:
                        ps = p1ps.tile([128, SC], F32, tag="p1", name="psqk")
                        if part == 0:
                            pending[key] = ps
                        dts = range(NDT) if part is None else range(step)
                    else:
                        ps = pending[key] if part < nparts - 1                             else pending.pop(key)
                        dts = range(part * step, (part + 1) * step)
                    for dt_i in dts:
                        nc.tensor.matmul(
                            ps[:],
                            w_sb[:, dt_i, cs],
                            x_sb[:, sc, :, dt_i, :],
                            start=(dt_i == 0), stop=(dt_i == NDT - 1),
                            skip_group_check=True,
                        )
                    if part is None or part == nparts - 1:
                        nc.vector.tensor_copy(dst[:, hp, ss], ps[:])

                def v_group(sc, sb):
                    ps = p1ps.tile([128, DLOC], F32, tag="p1", name="psv")
                    for dt_i in range(NDT):
                        nc.tensor.matmul(
                            ps[:],
                            x_sb[:, sc, sb, dt_i, :],
                            wv_sb[:, dt_i, :],
                            start=(dt_i == 0), stop=(dt_i == NDT - 1),
                            skip_group_check=True,
                        )
                    s_idx = sc * (SC // 128) + sb
                    nc.vector.tensor_copy(
                        vx[:, s_idx, :, 0:DH],
                        ps[:].rearrange("p (h d) -> p h d", h=HLOC))

                # -------- phase 1 head start: V (all) + K0 (all) + Q0 ----
                with nc.named_scope("qkv"):
                    for sc in range(NSC):
                        for sb in range(SC // 128):
                            v_group(sc, sb)
                        qk_group(0, wk_sb, kt, sc)
                    qk_group(0, wq_sb, qt, 0)

                # -------- phase 2: attention (QK for hp+1 interleaved) ---
                with nc.named_scope("attn"):
                    for hp in range(NHP):
                        for qc in range(NSC):
                            qs = slice(qc * SC, (qc + 1) * SC)
                            pvs = [pvp.tile([DH + 1, SC], F32, tag=f"pv{h}",
                                            name=f"pv{h}") for h in range(2)]
                            for kb in range(NKB):
                                ks = slice(kb * 128, (kb + 1) * 128)
                                sp = spp.tile([128, 2, SC], F32, tag="sp",
                                              name="sp")
                                for h in range(2):
                                    nc.tensor.matmul(
                                        sp[:, h, :],
                                        kt[64 * h:64 * h + 64, hp, ks],
                                        qt[64 * h:64 * h + 64, hp, qs],
                                        start=True, stop=True,
                                        tile_position=(64 * h, 0))
                                ex = exp_pool.tile([128, 2, SC], BF16,
                                                   tag="ex", name="ex")
                                nc.scalar.activation(ex[:], sp[:], EXPF,
                                                     scale=1.0 / H)
                                for h in range(2):
                                    nc.tensor.matmul(
                                        pvs[h][:], vx[:, kb, 2 * hp + h, :],
                                        ex[:, h, :],
                                        start=(kb == 0), stop=(kb == NKB - 1),
                                        skip_group_check=True)
                                if hp == 0 and qc < NSC - 1:
                                    if kb == 2:
                                        qk_group(0, wq_sb, qt, qc + 1,
                                                 part=0, nparts=2)
                                    elif kb == 4:
                                        qk_group(0, wq_sb, qt, qc + 1,
                                                 part=1, nparts=2)
                                if hp < NHP - 1:
                                    if kb == 7:
                                        qk_group(hp + 1, wk_sb, kt, qc,
                                                 part=0, nparts=2)
                                    elif kb == 9:
                                        qk_group(hp + 1, wk_sb, kt, qc,
                                                 part=1, nparts=2)
                                    elif kb == 12:
                                        qk_group(hp + 1, wq_sb, qt, qc,
                                                 part=0, nparts=2)
                                    elif kb == 14:
                                        qk_group(hp + 1, wq_sb, qt, qc,
                                                 part=1, nparts=2)
                            for h in range(2):
                                dr = nrm.tile([1, SC], F32, tag="dr",
                                              name="dr")
                                nc.vector.tensor_copy(dr[:],
                                                      pvs[h][DH:DH + 1, :])
                                pvc = nrm.tile([DH, SC], F32, tag="pvc",
                                               name="pvc")
                                nc.vector.tensor_copy(pvc[:],
                                                      pvs[h][0:DH, :])
                                den = nrm.tile([1, SC], F32, tag="den",
                                               name="den")
                                nc.vector.reciprocal_approx_fast(den[:],
                                                                 dr[:])
                                bc = nrm.tile([DH, SC], F32, tag="bc",
                                              name="bc")
                                nc.gpsimd.partition_broadcast(bc[:], den[:])
                                nc.vector.tensor_mul(
                                    ot[64 * h:64 * h + 64, hp, qs],
                                    pvc[:], bc[:])
                            nc.sync.dma_start(out_t[:, hp, qs],
                                              ot[:, hp, qs])

    nc.compile()
    return nc


def run(inputs, trace=False):
    x = np.asarray(inputs["encoder_input"], dtype=np.float32)
    Wq = np.asarray(inputs["Wq"], dtype=np.float32)
    Wk = np.asarray(inputs["Wk"], dtype=np.float32)
    Wv = np.asarray(inputs["Wv"], dtype=np.float32)
    bf = ml_dtypes.bfloat16

    nc = _build()
    in_maps = []
    for c in range(NCORES):
        b, g = c // 2, c % 2
        cols = slice(g * DLOC, (g + 1) * DLOC)
        xT = x[b].T                                       # [1024, 2048]
        x4 = (xT.reshape(NDT, 128, NSC, 4, 128)
              .transpose(1, 2, 3, 0, 4))
        in_maps.append({
            "x4": np.ascontiguousarray(x4.astype(bf)),
            "Wq": np.ascontiguousarray(
                Wq[:, cols].reshape(NDT, 128, DLOC).transpose(1, 0, 2)
                .astype(bf)),
            "Wk": np.ascontiguousarray(
                Wk[:, cols].reshape(NDT, 128, DLOC).transpose(1, 0, 2)
                .astype(bf)),
            "Wv": np.ascontiguousarray(
                Wv[:, cols].reshape(NDT, 128, DLOC).transpose(1, 0, 2)
                .astype(bf)),
        })
    res = run_bass_kernel_spmd(nc, in_maps, core_ids=list(range(NCORES)),
                               trace=trace)
    out = np.empty((B, S, D), dtype=np.float32)
    for c in range(NCORES):
        b, g = c // 2, c % 2
        out[b, :, g * DLOC:(g + 1) * DLOC] = res.results[c]["outT"].T
    return out, res


def kernel(**inputs):
    out, _ = run(inputs, trace=False)
    return out
